# revision 1
# baseline (speedup 1.0000x reference)
"""Trainium2 Bass kernel for a single-token GQA decoder layer (B=64 batches),
tensor-parallel across 8 NeuronCores.

Contract: kernel(**inputs) takes the FULL fp32 inputs (as produced by the
reference setup_inputs) and returns the FULL [64, 1, 4096] fp32 output.

Sharding (TP-8): core c owns q heads [4c, 4c+4), kv head c, MLP rows
[1792c, 1792(c+1)); hidden dim replicated. One on-device AllReduce after the
wo projection; the final down-proj partial sums are reduced on host.

Compute: bf16 matmul inputs (weights, KV cache, activations entering the PE),
fp32 accumulation/softmax/norms. Tiny precision-critical matmuls (rmsnorm
column sums, broadcast outer products) stay fp32.
"""

import numpy as np

import concourse.bass as bass
import concourse.bacc as bacc
import concourse.mybir as mybir
import concourse.tile as tile
from concourse.bass_utils import run_bass_kernel_spmd

FP = mybir.dt.float32
BF = mybir.dt.bfloat16
AX = mybir.AxisListType
AF = mybir.ActivationFunctionType
ALU = mybir.AluOpType

NCORES = 8
B = 64                    # batch (= tokens, QLEN=1)
DIM = 4096
HD = 128
G = 4                     # local q heads per core
S = 2048                  # prefix length
IL = 14336 // NCORES      # local intermediate = 1792
QKV = (G + 2) * HD        # 768 local qkv rows
EPS = 1e-6
QK_BF = True
PV_BF = True
GRP = 4                   # batches per attention score group (PSUM 32-part bands)
NGRP = B // GRP           # 16


def build_nc():
    nc = bacc.Bacc("TRN2", target_bir_lowering=False, debug=False,
                   num_devices=NCORES)

    # ---- DRAM I/O (per-core shards, host-prepped layouts) ----
    hs_d = nc.dram_tensor("hs", [B, DIM], FP, kind="ExternalInput")
    QB = BF if QK_BF else FP
    PB = BF if PV_BF else FP
    kT_d = nc.dram_tensor("kT", [B, HD, S], QB, kind="ExternalInput")
    vp_d = nc.dram_tensor("vp", [B, HD, S], PB, kind="ExternalInput")
    wqkvT_d = nc.dram_tensor("wqkvT", [DIM, QKV], BF, kind="ExternalInput")
    biasc_d = nc.dram_tensor("biasc", [HD, 6], FP, kind="ExternalInput")
    qnw_d = nc.dram_tensor("qnw", [1, HD], FP, kind="ExternalInput")
    knw_d = nc.dram_tensor("knw", [1, HD], FP, kind="ExternalInput")
    ones_d = nc.dram_tensor("ones128", [HD, 1], FP, kind="ExternalInput")
    id64_d = nc.dram_tensor("id64", [64, 64], BF, kind="ExternalInput")
    id128_d = nc.dram_tensor("id128", [128, 128], PB, kind="ExternalInput")
    woT_d = nc.dram_tensor("woT", [G * HD, DIM], BF, kind="ExternalInput")
    upT_d = nc.dram_tensor("upT", [DIM, IL], BF, kind="ExternalInput")
    gateT_d = nc.dram_tensor("gateT", [DIM, IL], BF, kind="ExternalInput")
    downT_d = nc.dram_tensor("downT", [IL, DIM], BF, kind="ExternalInput")

    partial_d = nc.dram_tensor("partial", [B, DIM], FP, kind="ExternalOutput")
    res2_d = nc.dram_tensor("res2", [B, DIM], FP, kind="ExternalOutput")

    with tile.TileContext(nc) as tc:
        with (
            tc.tile_pool(name="const", bufs=1) as constp,
            tc.tile_pool(name="sb", bufs=1) as sb,
            tc.tile_pool(name="str", bufs=3) as streams,      # kv stream tiles
            tc.tile_pool(name="wts", bufs=2) as wts,          # wqkv weight tiles
            tc.tile_pool(name="mlpw", bufs=8) as mlpw,       # [128,512] w tiles
            tc.tile_pool(name="pgrp", bufs=2) as pgrp,        # p / pT per group
            tc.tile_pool(name="small", bufs=4) as small,
            tc.tile_pool(name="ps_sc", bufs=1, space="PSUM") as ps_sc,
            tc.tile_pool(name="ps_stage", bufs=2, space="PSUM") as ps_stage,
            tc.tile_pool(name="ps_acc", bufs=2, space="PSUM") as ps_acc,
            tc.tile_pool(name="dram", bufs=1, space="DRAM") as dram,
        ):
            # ---- constants to SBUF ----
            id64 = constp.tile([64, 64], BF, tag="id64")
            nc.sync.dma_start(id64[:], id64_d[:])
            id128 = constp.tile([128, 128], PB, tag="id128")
            nc.sync.dma_start(id128[:], id128_d[:])
            ones128 = constp.tile([HD, 1], FP, tag="ones")
            nc.sync.dma_start(ones128[:], ones_d[:])
            qnw = constp.tile([1, HD], FP, tag="qnw")
            nc.sync.dma_start(qnw[:], qnw_d[:])
            knw = constp.tile([1, HD], FP, tag="knw")
            nc.sync.dma_start(knw[:], knw_d[:])
            biasc = constp.tile([HD, 6], FP, tag="biasc")
            nc.sync.dma_start(biasc[:], biasc_d[:])

            hs = sb.tile([B, DIM], FP, tag="hs")
            nc.sync.dma_start(hs[:], hs_d[:])

            # ================= helpers ==================================
            def rmsnorm_rstd(x_sb, tag):
                """rstd [64,1] fp32 for token-major x_sb [64, DIM]."""
                scr = sb.tile([B, DIM], FP, tag="big")
                ssq = small.tile([B, 1], FP, tag=tag + "ssq")
                nc.scalar.activation(scr[:], x_sb[:], AF.Square,
                                     accum_out=ssq[:])
                t1 = small.tile([B, 1], FP, tag=tag + "t1")
                nc.vector.tensor_scalar(t1[:], ssq[:], 1.0 / DIM, EPS,
                                        op0=ALU.mult, op1=ALU.add)
                rcp = small.tile([B, 1], FP, tag=tag + "rcp")
                nc.vector.reciprocal(rcp[:], t1[:])
                rstd = small.tile([B, 1], FP, tag=tag + "rstd")
                nc.scalar.activation(rstd[:], rcp[:], AF.Sqrt)
                return rstd

            def transpose_rows(x_sb, ncols, dest):
                """bf16 x_sb [64, ncols] -> bf16 dest [128, ncols//128*64]."""
                nch = ncols // 128
                for q in range(0, nch, 8):
                    hi = min(nch, q + 8)
                    stage = ps_stage.tile([128, 512], FP, tag="stage")
                    for j in range(q, hi):
                        nc.tensor.matmul(stage[:, (j - q) * 64:(j - q + 1) * 64],
                                         x_sb[:, j * 128:(j + 1) * 128],
                                         id64[:], start=True, stop=True)
                    nc.vector.tensor_copy(dest[:, q * 64:hi * 64],
                                          stage[:, 0:(hi - q) * 64])

            # ================= RMSNorm 1 + x^T ==========================
            rstd1 = rmsnorm_rstd(hs, "n1")
            x16 = sb.tile([B, DIM], BF, tag="x16")
            nc.vector.tensor_scalar_mul(x16[:], hs[:], rstd1[:])
            xT = sb.tile([128, B * DIM // 128], BF, tag="xT")   # [128, 2048]
            transpose_rows(x16, DIM, xT)

            # ================= QKV projection ===========================
            qkv_a = ps_acc.tile([B, 512], FP, tag="acc")
            qkv_b = ps_acc.tile([B, 256], FP, tag="acc")
            for j in range(32):
                wt = wts.tile([128, QKV], BF, tag="wqkv")
                nc.sync.dma_start(wt[:], wqkvT_d[j * 128:(j + 1) * 128, :])
                nc.tensor.matmul(qkv_a[:], xT[:, j * 64:(j + 1) * 64],
                                 wt[:, 0:512], start=(j == 0), stop=(j == 31))
                nc.tensor.matmul(qkv_b[:], xT[:, j * 64:(j + 1) * 64],
                                 wt[:, 512:768], start=(j == 0), stop=(j == 31))
            qkv_row = sb.tile([B, QKV], BF, tag="qkv_row")
            nc.vector.tensor_copy(qkv_row[:, 0:512], qkv_a[:])
            nc.vector.tensor_copy(qkv_row[:, 512:768], qkv_b[:])

            # transpose to [128 hd, 6*64] (fp32) and add bias
            qkvT = sb.tile([128, 6 * 64], FP, tag="qkvT")
            stage6 = ps_stage.tile([128, 512], FP, tag="stage")
            for c in range(6):
                nc.tensor.matmul(stage6[:, c * 64:(c + 1) * 64],
                                 qkv_row[:, c * 128:(c + 1) * 128],
                                 id64[:], start=True, stop=True)
            for c in range(6):
                nc.vector.tensor_scalar_add(qkvT[:, c * 64:(c + 1) * 64],
                                            stage6[:, c * 64:(c + 1) * 64],
                                            biasc[:, c:c + 1])

            # ================= q/k rmsnorm (over partition dim HD) ======
            sq2 = sb.tile([128, 320], FP, tag="sq2")
            nc.scalar.activation(sq2[:], qkvT[:, 0:320], AF.Square)
            ss = ps_stage.tile([1, 320], FP, tag="stage")
            nc.tensor.matmul(ss[:], ones128[:], sq2[:], start=True, stop=True)
            t2 = small.tile([1, 320], FP, tag="t2")
            nc.vector.tensor_scalar(t2[:], ss[:], 1.0 / HD, EPS,
                                    op0=ALU.mult, op1=ALU.add)
            rcp2 = small.tile([1, 320], FP, tag="rcp2")
            nc.vector.reciprocal(rcp2[:], t2[:])
            rstd2 = small.tile([1, 320], FP, tag="rstd2")
            nc.scalar.activation(rstd2[:], rcp2[:], AF.Sqrt)

            bq = ps_stage.tile([128, 256], FP, tag="stage")
            nc.tensor.matmul(bq[:], qnw[:], rstd2[0:1, 0:256],
                             start=True, stop=True)
            qn = sb.tile([128, 256], QB, tag="qn")
            nc.vector.tensor_tensor(qn[:], qkvT[:, 0:256], bq[:], op=ALU.mult)
            bk = ps_stage.tile([128, 64], FP, tag="stage")
            nc.tensor.matmul(bk[:], knw[:], rstd2[0:1, 256:320],
                             start=True, stop=True)
            kn = sb.tile([128, 64], QB, tag="kn")
            nc.vector.tensor_tensor(kn[:], qkvT[:, 256:320], bk[:], op=ALU.mult)

            # v_new rows [64 tok, 128] bf16
            v16 = sb.tile([128, 64], PB, tag="v16")
            nc.vector.tensor_copy(v16[:], qkvT[:, 320:384])
            vn_ps = ps_stage.tile([64, 128], FP, tag="stage")
            nc.tensor.matmul(vn_ps[:], v16[:], id128[:], start=True, stop=True)
            vnew = sb.tile([64, 128], PB, tag="vnew")
            nc.vector.tensor_copy(vnew[:], vn_ps[:])

            # q slices ordered [128, tok, g] (col = g*64 + tok)
            qn_r = qn[:].rearrange("p (g t) -> p t g", g=G)

            # ================= attention ================================
            # 4 batches/group, row(b, g) = 32*b + g (32-aligned PSUM bands).
            # sc rows outside the bands are never matmul-written; one memset
            # keeps them finite (they pass through exp/transposes unread).
            oT = sb.tile([128, B * G], BF, tag="oT")   # col = 16t + 4b + g
            sc = ps_sc.tile([128, S], FP, tag="sc")
            nc.vector.memset(sc[:], 0.0)
            for t in range(NGRP):
                last = ps_stage.tile([128, 1], FP, tag="stage")
                nc.vector.memset(last[:], 0.0)
                for b in range(GRP):
                    bg = t * GRP + b
                    kt = streams.tile([128, S], QB, tag="kv")
                    nc.sync.dma_start(kt[:], kT_d[bg])
                    for n in range(4):
                        nc.tensor.matmul(sc[32 * b:32 * b + 4,
                                            n * 512:(n + 1) * 512],
                                         qn_r[:, bg], kt[:, n * 512:(n + 1) * 512],
                                         start=True, stop=True,
                                         tile_position=(0, 32 * b))
                    nc.tensor.matmul(last[32 * b:32 * b + 4, 0:1],
                                     qn_r[:, bg], kn[:, bg:bg + 1],
                                     start=True, stop=True,
                                     tile_position=(0, 32 * b))
                # softmax over [128 rows = (b,g), 2048+1]
                nmax = small.tile([128, 1], FP, tag="nmax")
                nc.vector.reduce_max(nmax[:], sc[:], axis=AX.X, negate=True)
                lneg = small.tile([128, 1], FP, tag="lneg")
                nc.vector.tensor_scalar_mul(lneg[:], last[:], -1.0)
                negm = small.tile([128, 1], FP, tag="negm")
                nc.vector.tensor_tensor(negm[:], nmax[:], lneg[:], op=ALU.min)
                p_sb = pgrp.tile([128, S + 1], PB, tag="p")
                s1 = small.tile([128, 1], FP, tag="s1")
                nc.scalar.activation(p_sb[:, 0:S], sc[:], AF.Exp,
                                     bias=negm[:], accum_out=s1[:])
                plf = small.tile([128, 1], FP, tag="plf")
                nc.scalar.activation(plf[:], last[:], AF.Exp, bias=negm[:])
                nc.vector.tensor_copy(p_sb[:, S:S + 1], plf[:])
                stot = small.tile([128, 1], FP, tag="stot")
                nc.vector.tensor_tensor(stot[:], s1[:], plf[:], op=ALU.add)
                rs = small.tile([128, 1], FP, tag="rs")
                nc.vector.reciprocal(rs[:], stot[:])

                # transpose p -> pT [128 seq, col = 32b+g] in 16 chunks
                pT = pgrp.tile([128, 16 * 128], PB, tag="pT")
                for q in range(0, 16, 4):
                    stage = ps_stage.tile([128, 512], FP, tag="stage")
                    for j in range(q, q + 4):
                        nc.tensor.matmul(stage[:, (j - q) * 128:(j - q + 1) * 128],
                                         p_sb[:, j * 128:(j + 1) * 128],
                                         id128[:], start=True, stop=True)
                    nc.vector.tensor_copy(pT[:, q * 128:(q + 4) * 128],
                                          stage[:])
                pl_ps = ps_stage.tile([1, 128], FP, tag="stage")
                nc.tensor.matmul(pl_ps[:], p_sb[:, S:S + 1], id128[:],
                                 start=True, stop=True)
                plast = small.tile([1, 128], PB, tag="plast")
                nc.vector.tensor_copy(plast[:], pl_ps[:])

                # PV: o[32b+g, hd] accumulated per batch band
                o_ps = ps_stage.tile([128, 128], FP, tag="stage")
                nc.vector.memset(o_ps[:], 0.0)
                for b in range(GRP):
                    bg = t * GRP + b
                    vt = streams.tile([128, S], PB, tag="kv2")
                    nc.sync.dma_start(vt[:], vp_d[bg])
                    for j in range(16):
                        nc.tensor.matmul(o_ps[32 * b:32 * b + 4, :],
                                         pT[:, j * 128 + 32 * b:j * 128 + 32 * b + 4],
                                         vt[:, j * 128:(j + 1) * 128],
                                         start=(j == 0), stop=False,
                                         tile_position=(0, 32 * b))
                    vne = small.tile([1, 128], PB, tag="vne")
                    nc.sync.dma_start(vne[:], vnew[bg:bg + 1, :])
                    nc.tensor.matmul(o_ps[32 * b:32 * b + 4, :],
                                     plast[0:1, 32 * b:32 * b + 4],
                                     vne[:],
                                     start=False, stop=True,
                                     tile_position=(0, 32 * b))
                o_row = sb.tile([128, 128], PB, tag="o_row")
                nc.vector.tensor_scalar_mul(o_row[:], o_ps[:], rs[:])
                # full base-0 transpose (bf16 FWL + row tile_position faults),
                # then copy only the 16 valid cols (32b+g) out of 128.
                oT_ps = ps_stage.tile([128, 128], FP, tag="stage")
                nc.tensor.matmul(oT_ps[:], o_row[:], id128[:],
                                 start=True, stop=True)
                oT_v = oT_ps[:].rearrange("p (b x) -> p b x", b=GRP)
                nc.vector.tensor_copy(
                    oT[:, t * 16:(t + 1) * 16].rearrange(
                        "p (b g) -> p b g", b=GRP),
                    oT_v[:, :, 0:G])

            # ================= wo projection ============================
            # contraction over (g, d) = 4 chunks of 128; lhsT cols: token order
            oT_r = oT[:].rearrange("p (t b g) -> p g t b", t=NGRP, g=G)
            cc_in = dram.tile([B, DIM], FP)
            cc_out = dram.tile([B, DIM], FP)
            for n in range(8):
                wo_ps = ps_acc.tile([B, 512], FP, tag="acc")
                for kk in range(4):
                    wt = mlpw.tile([128, 512], BF, tag="mw")
                    nc.sync.dma_start(wt[:], woT_d[kk * 128:(kk + 1) * 128,
                                                   n * 512:(n + 1) * 512])
                    nc.tensor.matmul(wo_ps[:], oT_r[:, kk], wt[:],
                                     start=(kk == 0), stop=(kk == 3))
                stg = small.tile([B, 512], FP, tag="ostg")
                nc.vector.tensor_copy(stg[:], wo_ps[:])
                nc.sync.dma_start(cc_in[:, n * 512:(n + 1) * 512], stg[:])

            # ================= AllReduce + residual =====================
            nc.gpsimd.collective_compute(
                "AllReduce", ALU.add,
                replica_groups=[list(range(NCORES))],
                ins=[cc_in[:].opt()], outs=[cc_out[:].opt()],
            )
            ar = sb.tile([B, DIM], FP, tag="big")
            nc.sync.dma_start(ar[:], cc_out[:])
            hidden = sb.tile([B, DIM], FP, tag="hidden")
            nc.vector.tensor_tensor(hidden[:], hs[:], ar[:], op=ALU.add)
            nc.sync.dma_start(res2_d[:], hidden[:])

            # ================= RMSNorm 2 + MLP ==========================
            rstd2h = rmsnorm_rstd(hidden, "n2")
            h16 = sb.tile([B, DIM], BF, tag="x16")
            nc.vector.tensor_scalar_mul(h16[:], hidden[:], rstd2h[:])
            hT = sb.tile([128, B * DIM // 128], BF, tag="xT")
            transpose_rows(h16, DIM, hT)

            g_row = sb.tile([B, IL], FP, tag="g_row")
            gu_row = sb.tile([B, IL], BF, tag="gu_row")
            nch = [(0, 512), (512, 512), (1024, 512), (1536, 256)]
            for (c0, cw) in nch:
                up_ps = ps_acc.tile([B, 512], FP, tag="acc")
                gt_ps = ps_acc.tile([B, 512], FP, tag="acc")
                for j in range(32):
                    uw = mlpw.tile([128, 512], BF, tag="mw")
                    nc.sync.dma_start(uw[:, 0:cw], upT_d[j * 128:(j + 1) * 128,
                                                         c0:c0 + cw])
                    gw = mlpw.tile([128, 512], BF, tag="mw")
                    nc.sync.dma_start(gw[:, 0:cw], gateT_d[j * 128:(j + 1) * 128,
                                                           c0:c0 + cw])
                    nc.tensor.matmul(up_ps[:, 0:cw], hT[:, j * 64:(j + 1) * 64],
                                     uw[:, 0:cw], start=(j == 0), stop=(j == 31))
                    nc.tensor.matmul(gt_ps[:, 0:cw], hT[:, j * 64:(j + 1) * 64],
                                     gw[:, 0:cw], start=(j == 0), stop=(j == 31))
                nc.scalar.activation(g_row[:, c0:c0 + cw], gt_ps[:, 0:cw],
                                     AF.Silu)
                nc.vector.tensor_tensor(gu_row[:, c0:c0 + cw], up_ps[:, 0:cw],
                                        g_row[:, c0:c0 + cw], op=ALU.mult)

            guT = sb.tile([128, 14 * 64], BF, tag="guT")
            transpose_rows(gu_row, IL, guT)

            for n in range(8):
                dn_ps = ps_acc.tile([B, 512], FP, tag="acc")
                for c in range(14):
                    dw = mlpw.tile([128, 512], BF, tag="mw")
                    nc.sync.dma_start(dw[:], downT_d[c * 128:(c + 1) * 128,
                                                     n * 512:(n + 1) * 512])
                    nc.tensor.matmul(dn_ps[:], guT[:, c * 64:(c + 1) * 64],
                                     dw[:], start=(c == 0), stop=(c == 13))
                stg = small.tile([B, 512], FP, tag="ostg")
                nc.vector.tensor_copy(stg[:], dn_ps[:])
                nc.sync.dma_start(partial_d[:, n * 512:(n + 1) * 512], stg[:])

    nc.compile()
    return nc


def shard_inputs(inputs):
    """Full fp32 inputs -> list of 8 per-core input maps (host prep)."""
    f32 = np.float32
    bf16 = mybir.dt.np(BF)
    hs = np.ascontiguousarray(inputs["hidden_states"].reshape(B, DIM), f32)
    wqkv = np.asarray(inputs["wqkv_w"], f32)
    wb = np.asarray(inputs["wqkv_b"], f32)
    wo = np.asarray(inputs["wo_w"], f32)
    up = np.asarray(inputs["up_w"], f32)
    gate = np.asarray(inputs["gate_w"], f32)
    down = np.asarray(inputs["down_w"], f32)
    qnorm = np.asarray(inputs["qnorm_w"], f32)
    knorm = np.asarray(inputs["knorm_w"], f32)
    iln = np.asarray(inputs["in_ln_w"], f32)
    pln = np.asarray(inputs["post_ln_w"], f32)
    kc = np.asarray(inputs["k_cache"], f32)   # [B, S, 8, HD]
    vc = np.asarray(inputs["v_cache"], f32)

    id64 = np.eye(64, dtype=bf16)
    qb = mybir.dt.np(BF) if QK_BF else np.float32
    pb = mybir.dt.np(BF) if PV_BF else np.float32
    id128 = np.eye(128, dtype=pb)
    ones128 = np.ones((HD, 1), f32)
    qnw = (qnorm / np.sqrt(HD)).reshape(1, HD).astype(f32)
    knw = knorm.reshape(1, HD).astype(f32)

    H = 32
    maps = []
    for c in range(NCORES):
        wq = wqkv[c * G * HD:(c + 1) * G * HD]              # [512, DIM]
        wk = wqkv[H * HD + c * HD:H * HD + (c + 1) * HD]    # [128, DIM]
        wv = wqkv[(H + 8) * HD + c * HD:(H + 8) * HD + (c + 1) * HD]
        wloc = np.concatenate([wq, wk, wv], axis=0)         # [768, DIM]
        wqkvT = np.ascontiguousarray((wloc * iln[None, :]).T.astype(bf16))
        bq = wb[c * G * HD:(c + 1) * G * HD]
        bk = wb[H * HD + c * HD:H * HD + (c + 1) * HD]
        bv = wb[(H + 8) * HD + c * HD:(H + 8) * HD + (c + 1) * HD]
        biasc = np.ascontiguousarray(
            np.concatenate([bq, bk, bv]).reshape(6, HD).T)  # [128, 6]

        kT = np.ascontiguousarray(
            kc[:, :, c, :].transpose(0, 2, 1).astype(qb))  # [B,HD,S]
        vp = np.ascontiguousarray(
            vc[:, :, c, :].reshape(B, 16, 128, HD).transpose(0, 2, 1, 3)
            .astype(pb)).reshape(B, HD, S)

        woT = np.ascontiguousarray(
            wo[:, c * G * HD:(c + 1) * G * HD].T.astype(bf16))
        upT = np.ascontiguousarray(
            (up[c * IL:(c + 1) * IL] * pln[None, :]).T.astype(bf16))
        gateT = np.ascontiguousarray(
            (gate[c * IL:(c + 1) * IL] * pln[None, :]).T.astype(bf16))
        downT = np.ascontiguousarray(down[:, c * IL:(c + 1) * IL].T.astype(bf16))

        maps.append({
            "hs": hs, "kT": kT, "vp": vp, "wqkvT": wqkvT, "biasc": biasc,
            "qnw": qnw, "knw": knw, "ones128": ones128, "id64": id64,
            "id128": id128, "woT": woT, "upT": upT, "gateT": gateT,
            "downT": downT,
        })
    return maps


_NC = None


def _get_nc():
    global _NC
    if _NC is None:
        _NC = build_nc()
    return _NC


def run(inputs, **kw):
    nc = _get_nc()
    in_maps = shard_inputs(inputs)
    res = run_bass_kernel_spmd(nc, in_maps, list(range(NCORES)), **kw)
    out = res.results[0]["res2"].astype(np.float64)
    for c in range(NCORES):
        out = out + res.results[c]["partial"].astype(np.float64)
    return out.astype(np.float32).reshape(B, 1, DIM), res


def kernel(**inputs):
    out, _ = run(inputs)
    return out

